# revision 13
# baseline (speedup 1.0000x reference)
"""Trainium2 Bass kernel for nn_Axon_53489522704543 (scatter_memory).

Computation (reference):
    att = clip(attenuation, 0, 1); decay = 0.9**delays
    signals[b,s,br] = spikes[b,s] * att[s,br] * decay[s,br]
    out[b,t] = sum over (s,br) with target_indices[s,br]==t of signals[b,s,br]

Strategy: source-parallel over 8 cores (2048 sources each). Per core the
scatter matrix A[s, t] = sum_br W[s,br] * one_hot(t[s,br]) (W = att*decay)
is materialized window-by-window as dense bf16 strips with GPSIMD
local_scatter (per-partition indexed writes; indices are host-prepared
int16), then contracted on the PE:

    out[t, b] = sum_s A[s, t] * spk[s, b]

Targets are split into 16 windows of 1024; sources into 16 tiles of 128.
For each (window, s_tile): one local_scatter builds the [128, 1024] strip;
8 matmuls of [128,128]^T @ [128,32] accumulate psum[t_loc, b] over s_tiles.

Pairs that cannot ride a strip — duplicate (source,target) pairs (W values
would need merging; local_scatter forbids duplicate indices) and per-
(source,window) rank overflow beyond NI slots — go through a small exact
PE path: a one-hot source gather (P[s,i] = (src_i==s)) pulls their spike
rows into psum, v = gathered * W, and a one-hot t_loc matmul scatters them
into the same psum accumulation before `stop`.

Host does index-only preprocessing (padding/permutation of weight and index
arrays) plus the final gather: sum of 8 per-core [16384, 32] partials.
"""

import numpy as np
import ml_dtypes

import concourse.bacc as bacc
import concourse.bass as bass
import concourse.mybir as mybir
import concourse.tile as tile
from concourse.alu_op_type import AluOpType
from concourse.bass_utils import run_bass_kernel_spmd

N_CORES = 8
S = 16384          # sources
T = 16384          # targets
BR = 64            # branches
B = 32             # batch
SC = S // N_CORES  # sources per core (2048)
NW = 16            # target windows of 1024
NK = 16            # source tiles of 128
NE = 1024          # targets per window
NI = 16            # strip slots per (source, window)
LTW = 2            # leftover tiles (of 128 pairs) per window
SMOOTHING = 0.9

F32 = mybir.dt.float32
BF16 = mybir.dt.bfloat16
I16 = mybir.dt.int16

_CACHE = {}
REPEAT = 1  # >1: wrap the compute loop in For_i for timing measurements


def _build():
    nc = bacc.Bacc("TRN2", target_bir_lowering=False, debug=False,
                   num_devices=N_CORES)

    spk_d = nc.dram_tensor("spk", [128, NK * B], BF16, kind="ExternalInput")
    att_d = nc.dram_tensor("att", [128, NW * NK * NI], F32, kind="ExternalInput")
    dly_d = nc.dram_tensor("dly", [128, NW * NK * NI], F32, kind="ExternalInput")
    lsi_d = nc.dram_tensor("lsi", [128, NW * NK * NI], I16, kind="ExternalInput")
    srcb_d = nc.dram_tensor("srcb", [128, NW * LTW * 128], F32, kind="ExternalInput")
    ltl_d = nc.dram_tensor("ltl", [128, NW * LTW], F32, kind="ExternalInput")
    latt_d = nc.dram_tensor("latt", [128, NW * LTW], F32, kind="ExternalInput")
    ldly_d = nc.dram_tensor("ldly", [128, NW * LTW], F32, kind="ExternalInput")
    iot_d = nc.dram_tensor("iot", [128, 128], F32, kind="ExternalInput")
    iotc_d = nc.dram_tensor("iotc", [128, 1], F32, kind="ExternalInput")
    part_d = nc.dram_tensor("part", [T, B], F32, kind="ExternalOutput")

    with tile.TileContext(nc) as tc:
        with (
            tc.tile_pool(name="slab", bufs=1) as slab,
            tc.tile_pool(name="wtmp", bufs=1) as wtmp,
            tc.tile_pool(name="strips", bufs=12) as strips,
            tc.tile_pool(name="oh", bufs=3) as ohp,
            tc.tile_pool(name="psum", bufs=2, space="PSUM") as psp,
        ):
            spk_t = slab.tile([128, NK * B], BF16, tag="spk")
            att_t = slab.tile([128, NW * NK * NI], F32, tag="att")
            dly_t = slab.tile([128, NW * NK * NI], F32, tag="dly")
            lsi_t = slab.tile([128, NW * NK * NI], I16, tag="lsi")
            srcb_t = slab.tile([128, NW * LTW * 128], F32, tag="srcb")
            ltl_t = slab.tile([128, NW * LTW], F32, tag="ltl")
            latt_t = slab.tile([128, NW * LTW], F32, tag="latt")
            ldly_t = slab.tile([128, NW * LTW], F32, tag="ldly")
            iot_t = slab.tile([128, 128], F32, tag="iot")
            iotc_t = slab.tile([128, 1], F32, tag="iotc")
            wb_t = slab.tile([128, NW * NK * NI], BF16, tag="wb")
            wl_t = slab.tile([128, NW * LTW], F32, tag="wl")
            out_t = slab.tile([128, NW * 8 * B], F32, tag="out")
            zlhs_t = slab.tile([128, 128], BF16, tag="zlhs")
            zrhs_t = slab.tile([128, B], BF16, tag="zrhs")

            nc.vector.memset(zlhs_t[:], 0.0)
            nc.vector.memset(zrhs_t[:], 0.0)
            for t_, d_ in [(spk_t, spk_d), (att_t, att_d), (dly_t, dly_d),
                           (lsi_t, lsi_d), (srcb_t, srcb_d), (ltl_t, ltl_d),
                           (latt_t, latt_d), (ldly_t, ldly_d), (iot_t, iot_d),
                           (iotc_t, iotc_d)]:
                nc.sync.dma_start(t_[:], d_.ap())

            import contextlib
            rep_ctx = (tc.For_i(0, REPEAT, 1) if REPEAT > 1
                       else contextlib.nullcontext())
            with rep_ctx:
                # W = clip(att,0,1) * 0.9^dly (exact 6-term one-hot decay),
                # for both the strip slots and the leftover columns.
                for src_att, src_dly, dst, dstdt in [
                        (att_t, dly_t, wb_t, BF16), (latt_t, ldly_t, wl_t, F32)]:
                    n = src_att.shape[1]
                    w_f = wtmp.tile([128, n], F32, tag=f"wf{n}")
                    dec = wtmp.tile([128, n], F32, tag=f"dec{n}")
                    trm = wtmp.tile([128, n], F32, tag=f"trm{n}")
                    nc.vector.tensor_scalar(w_f[:], src_att[:], 0.0, 1.0,
                                            AluOpType.max, AluOpType.min)
                    for k in range(6):
                        d = dec if k == 0 else trm
                        nc.vector.tensor_scalar(d[:], src_dly[:], float(k),
                                                float(SMOOTHING ** k),
                                                AluOpType.is_equal,
                                                AluOpType.mult)
                        if k > 0:
                            nc.vector.tensor_tensor(dec[:], dec[:], trm[:],
                                                    AluOpType.add)
                    nc.vector.tensor_tensor(dst[:], w_f[:], dec[:],
                                            AluOpType.mult)

                for w in range(NW):
                    ps = psp.tile([128, 8 * B], F32)      # [t_loc128 x (tb, b)]
                    psg = psp.tile([128, LTW * B], F32)   # leftover spike gather

                    # start=True matmuls zero the psum region but their own
                    # product does not land; issue sacrificial zero-product
                    # start matmuls, then accumulate everything start=False.
                    for tb in range(8):
                        nc.tensor.matmul(ps[:, tb * B:(tb + 1) * B],
                                         zlhs_t[:], zrhs_t[:],
                                         start=True, stop=False)
                    for lt in range(LTW):
                        nc.tensor.matmul(psg[:, lt * B:(lt + 1) * B],
                                         zlhs_t[:], zrhs_t[:],
                                         start=True, stop=False)

                    for k in range(NK):
                        call = w * NK + k
                        strip = strips.tile([128, NE], BF16, tag="strip")
                        # Same-engine fence writes on one col per 128-block
                        # around the scatter: the pre-fence WAR-stalls Pool
                        # until the buffer's previous reader matmuls retire,
                        # the post-fence RAW-gates consumer matmuls; keeps
                        # Pool and PE in lockstep (measurably faster too).
                        sap = strip[:]
                        fence = bass.AP(sap.tensor, sap.offset,
                                        [[sap.ap[0][0], 128], [128, 8]])
                        nc.gpsimd.tensor_scalar(fence, fence, 1.0, None,
                                                AluOpType.mult)
                        nc.gpsimd.local_scatter(
                            strip[:], wb_t[:, call * NI:(call + 1) * NI],
                            lsi_t[:, call * NI:(call + 1) * NI], 128, NE, NI)
                        nc.gpsimd.tensor_scalar(fence, fence, 1.0, None,
                                                AluOpType.mult)
                        for tb in range(8):
                            nc.tensor.matmul(
                                ps[:, tb * B:(tb + 1) * B],
                                strip[:, tb * 128:(tb + 1) * 128],
                                spk_t[:, k * B:(k + 1) * B],
                                start=False, stop=False)

                    # Leftover pairs of this window: one-hot gather of their
                    # spike rows, scale by W, one-hot scatter into ps.
                    vl = ohp.tile([128, LTW * B], BF16, tag="vl")
                    for lt in range(LTW):
                        col = w * LTW + lt
                        for k in range(NK):
                            pg = ohp.tile([128, 128], BF16, tag="pg")
                            nc.vector.tensor_scalar(
                                pg[:], srcb_t[:, col * 128:(col + 1) * 128],
                                float(128 * k), iotc_t[:, 0:1],
                                AluOpType.subtract, AluOpType.is_equal)
                            nc.tensor.matmul(
                                psg[:, lt * B:(lt + 1) * B], pg[:],
                                spk_t[:, k * B:(k + 1) * B],
                                start=False, stop=(k == NK - 1))
                        nc.vector.tensor_scalar(
                            vl[:, lt * B:(lt + 1) * B],
                            psg[:, lt * B:(lt + 1) * B],
                            wl_t[:, col:col + 1], None,
                            AluOpType.mult)
                    for lt in range(LTW):
                        col = w * LTW + lt
                        for tb in range(8):
                            oh = ohp.tile([128, 128], BF16, tag="oh")
                            nc.vector.tensor_scalar(
                                oh[:], iot_t[:], float(tb * 128),
                                ltl_t[:, col:col + 1],
                                AluOpType.add, AluOpType.is_equal)
                            nc.tensor.matmul(
                                ps[:, tb * B:(tb + 1) * B], oh[:],
                                vl[:, lt * B:(lt + 1) * B],
                                start=False, stop=(lt == LTW - 1))

                    nc.vector.tensor_copy(out_t[:, w * 8 * B:(w + 1) * 8 * B],
                                          ps[:])

            nc.sync.dma_start(
                bass.AP(part_d, 0, [[B, 128], [128 * B, NW * 8], [1, B]]),
                out_t[:])

    nc.compile()
    return nc


def prepare_in_maps(spikes, attenuation, target_indices, delays):
    spikes = np.asarray(spikes, dtype=np.float32)
    att = np.asarray(attenuation, dtype=np.float32)
    tgt = np.asarray(target_indices).astype(np.int64)
    dly = np.asarray(delays).astype(np.float32)
    spikesT = np.ascontiguousarray(spikes.T)                   # [S, B]

    iot = np.broadcast_to(np.arange(128, dtype=np.float32), (128, 128)).copy()
    iotc = np.arange(128, dtype=np.float32)[:, None].copy()

    in_maps = []
    for c in range(N_CORES):
        sl = slice(c * SC, (c + 1) * SC)
        A = att[sl]
        D = dly[sl]
        Tg = tgt[sl]
        N = SC * BR

        s_idx = np.repeat(np.arange(SC), BR)
        t = Tg.reshape(-1)
        a_f = A.reshape(-1)
        d_f = D.reshape(-1)
        w = t >> 10
        tl = (t & 1023).astype(np.int64)

        # duplicate (source, target) pairs: non-first occurrences
        pk = s_idx * T + t
        order = np.argsort(pk, kind="stable")
        dup_sorted = np.r_[False, pk[order][1:] == pk[order][:-1]]
        dup = np.zeros(N, bool)
        dup[order] = dup_sorted

        # rank of non-dup pairs within their (source, window) group
        gk = np.where(~dup, s_idx * NW + w, 1 << 40)
        order2 = np.argsort(gk, kind="stable")
        gs = gk[order2]
        first = np.r_[True, gs[1:] != gs[:-1]]
        startpos = np.flatnonzero(first)
        gid = np.cumsum(first) - 1
        rank = np.empty(N, np.int64)
        rank[order2] = np.arange(N) - startpos[gid]

        main = ~dup & (rank < NI)
        left = dup | (~dup & (rank >= NI))

        datt = np.zeros((128, NW * NK, NI), np.float32)
        ddly = np.zeros((128, NW * NK, NI), np.float32)
        didx = np.full((128, NW * NK, NI), -1, np.int16)
        p = s_idx % 128
        kk = s_idx // 128
        call = w * NK + kk
        didx[p[main], call[main], rank[main]] = tl[main]
        datt[p[main], call[main], rank[main]] = a_f[main]
        ddly[p[main], call[main], rank[main]] = d_f[main]

        srcb = np.zeros((128, NW * LTW, 128), np.float32)
        ltl = np.zeros((128, NW * LTW), np.float32)
        latt = np.zeros((128, NW * LTW), np.float32)
        ldly = np.zeros((128, NW * LTW), np.float32)
        li_all = np.flatnonzero(left)
        for wi in range(NW):
            li = li_all[w[li_all] == wi]
            assert len(li) <= 128 * LTW, (c, wi, len(li))
            q = np.arange(len(li))
            lt, pp = q // 128, q % 128
            cols = wi * LTW + lt
            srcb[:, cols, pp] = s_idx[li]
            ltl[pp, cols] = tl[li]
            latt[pp, cols] = a_f[li]
            ldly[pp, cols] = d_f[li]

        spk_slab = (spikesT[sl].reshape(NK, 128, B).transpose(1, 0, 2)
                    .reshape(128, NK * B).astype(ml_dtypes.bfloat16))

        in_maps.append({
            "spk": np.ascontiguousarray(spk_slab),
            "att": datt.reshape(128, -1),
            "dly": ddly.reshape(128, -1),
            "lsi": didx.reshape(128, -1),
            "srcb": srcb.reshape(128, -1),
            "ltl": ltl, "latt": latt, "ldly": ldly,
            "iot": iot, "iotc": iotc,
        })
    return in_maps


def kernel(spikes, attenuation, target_indices, delays):
    if "nc" not in _CACHE:
        _CACHE["nc"] = _build()
    nc = _CACHE["nc"]

    in_maps = prepare_in_maps(spikes, attenuation, target_indices, delays)
    res = run_bass_kernel_spmd(nc, in_maps, core_ids=list(range(N_CORES)))
    _CACHE["last_result"] = res

    acc = np.zeros((T, B), dtype=np.float64)
    for c in range(N_CORES):
        acc += res.results[c]["part"].astype(np.float64)
    return np.ascontiguousarray(acc.T).astype(np.float32)
